# revision 12
# baseline (speedup 1.0000x reference)
"""Trainium2 Bass kernel for nn_BaseEmbedder (retrieval_knn).

For each of 4096 query embeddings: find the 5 nearest of 65536 db embeddings
(Euclidean) and produce the inverse-distance-weighted sum of their auxiliary
features.  SPMD on 8 NeuronCores: queries sharded 512/core, db+aux replicated.

Per core (512 queries = 4 q-tiles of 128 partitions):
  - Scan (bf16): negS[q,j] = q.x_j - 0.5|x_j|^2 via K=34 augmented bf16
    matmuls (rows 32/33 carry the -0.5|x|^2 bias split hi/lo).  Two matmul
    streams run concurrently on PE row-groups 0 and 64 (even/odd 1024-col
    supers).
  - Pair-fold: DVE tensor_tensor(max) folds each super pair (PSUM A operand,
    SBUF copy of B) into zfold[u] = max(y_even[u], y_odd[u]) - 8192 values
    per 16384-column window.
  - Candidates: per window, max8 over zfold gives the top-8 folded values;
    max_index recovers their fold slots (needles are window-local, exact f32
    match).  Each slot maps to TWO db rows (the fold pair); both are
    gathered, so no parity disambiguation is needed.
  - Exact refinement (f32): a host-prepared paired table
    row[w*8192+p*1024+u] = [x_j1, aux_j1, x_j2, aux_j2, |x_j1|^2, |x_j2|^2]
    is gathered per winning slot via indirect DMA (one row per candidate
    pair per query).  Exact distances for all 64 candidates are recomputed
    on-chip; top-5 by threshold; weights 1/(d+eps) normalized; weighted sum.

The bf16 scan only nominates candidates; all selection/weight math is exact
f32, so the result matches the f32 reference to ~1e-6.
"""

import numpy as np
import ml_dtypes

from concourse import bass, mybir
from concourse.tile import TileContext
from concourse.bass_utils import run_bass_kernel_spmd

F32 = mybir.dt.float32
BF16 = mybir.dt.bfloat16
U32 = mybir.dt.uint32
I32 = mybir.dt.int32

N_CORES = 8
NQ = 4096
NDB = 65536
D = 32
DAUG = 34   # 32 dims + bias row + bias-residual row (bf16 split)
K = 5
EPS = 1e-6

NQ_CORE = NQ // N_CORES          # 512
CHUNK = 512                      # db columns per matmul (one PSUM bank)
SUPER = 1024                     # db columns per PSUM tile / fold operand
WINDOW = 32768                   # raw db columns per max8 + max_index window
RG_B = 64                        # partition base of the second PE row-group
PV = 132                         # paired-table row: xA,auxA,xB,auxB,xsqA,xsqB,pad


def build_nc(nq_core=NQ_CORE, ndb=NDB):
    n_qt = nq_core // 128
    win = min(WINDOW, ndb)
    n_win = ndb // win
    n_pair = win // (2 * SUPER)          # fold pairs per window (8)
    fold_w = win // 2                    # folded columns per window (8192)
    npair_c = 8 * n_win                  # candidate pairs per query
    ncand = 2 * npair_c                  # candidates per query

    nc = bass.Bass()
    qT = nc.declare_dram_parameter("qT_aug", [DAUG, nq_core], BF16, isOutput=False)
    qf = nc.declare_dram_parameter("qf", [nq_core, D], F32, isOutput=False)
    qsq = nc.declare_dram_parameter("qsq", [nq_core, 1], F32, isOutput=False)
    dbT = nc.declare_dram_parameter("dbT_aug", [DAUG, ndb], BF16, isOutput=False)
    pairt = nc.declare_dram_parameter("pair_table", [ndb // 2, PV], F32,
                                      isOutput=False)
    out = nc.declare_dram_parameter("out", [nq_core, D], F32, isOutput=True)

    with TileContext(nc) as tc:
        with (
            tc.tile_pool(name="zf", bufs=1) as zfp,
            tc.tile_pool(name="db", bufs=3) as dbp,
            tc.tile_pool(name="sbB", bufs=3) as sbp,
            tc.tile_pool(name="psA", bufs=2, space="PSUM") as pspA,
            tc.tile_pool(name="psB", bufs=2, space="PSUM") as pspB,
            tc.tile_pool(name="sm", bufs=2) as sp,
            tc.tile_pool(name="g", bufs=2) as gp,
        ):
            for t in range(n_qt):
                # queries live on row groups 0 and RG_B so two matmul streams
                # run concurrently on the PE array
                qt = sp.tile([128, 128], BF16, tag="qt")
                nc.sync.dma_start(out=qt[0:DAUG, :],
                                  in_=qT[:, t * 128:(t + 1) * 128])
                nc.sync.dma_start(out=qt[RG_B:RG_B + DAUG, :],
                                  in_=qT[:, t * 128:(t + 1) * 128])
                qs = sp.tile([128, 1], F32, tag="qs")
                nc.sync.dma_start(out=qs[:], in_=qsq[t * 128:(t + 1) * 128, :])
                qft = sp.tile([128, D], F32, tag="qft")
                nc.sync.dma_start(out=qft[:], in_=qf[t * 128:(t + 1) * 128, :])

                candv = sp.tile([128, npair_c], F32, tag="candv")
                gxa = gp.tile([128, npair_c, PV], F32, tag="gxa")

                for w in range(n_win):
                    zfold = zfp.tile([128, fold_w], F32)
                    for p in range(n_pair):
                        offa = w * win + (2 * p) * SUPER
                        rhs = dbp.tile([128, SUPER], BF16)
                        nc.sync.dma_start(out=rhs[0:DAUG, :],
                                          in_=dbT[:, offa:offa + SUPER])
                        nc.sync.dma_start(
                            out=rhs[RG_B:RG_B + DAUG, :],
                            in_=dbT[:, offa + SUPER:offa + 2 * SUPER])
                        psA = pspA.tile([128, SUPER], F32, tag="psA")
                        psB = pspB.tile([128, SUPER], F32, tag="psB")
                        for m in range(SUPER // CHUNK):
                            sl = slice(m * CHUNK, (m + 1) * CHUNK)
                            nc.tensor.matmul(out=psA[:, sl],
                                             lhsT=qt[0:DAUG, :],
                                             rhs=rhs[0:DAUG, sl],
                                             start=True, stop=True,
                                             tile_position=(0, 0))
                            nc.tensor.matmul(out=psB[:, sl],
                                             lhsT=qt[RG_B:RG_B + DAUG, :],
                                             rhs=rhs[RG_B:RG_B + DAUG, sl],
                                             start=True, stop=True,
                                             tile_position=(RG_B, 0))
                        sbB = sbp.tile([128, SUPER], F32)
                        nc.scalar.copy(out=sbB[:], in_=psB[:])
                        nc.vector.tensor_tensor(
                            out=zfold[:, p * SUPER:(p + 1) * SUPER],
                            in0=psA[:], in1=sbB[:], op=mybir.AluOpType.max)
                    # window top-8 folded values + their fold slots
                    w8 = candv[:, w * 8:(w + 1) * 8]
                    nc.vector.max(out=w8, in_=zfold[:])
                    pos = sp.tile([128, 8], U32, tag="pos")
                    nc.vector.max_index(out=pos[:], in_max=w8,
                                        in_values=zfold[:])
                    # paired-table row = slot + w*fold_w; gather immediately
                    ji = sp.tile([128, 8], I32, tag="ji")
                    nc.vector.tensor_scalar_add(ji[:], pos[:],
                                                float(w * fold_w))
                    for i in range(8):
                        nc.gpsimd.indirect_dma_start(
                            out=gxa[:, w * 8 + i, :], out_offset=None,
                            in_=pairt[:],
                            in_offset=bass.IndirectOffsetOnAxis(
                                ap=ji[:, i:i + 1], axis=0))

                # ---- exact f32 refinement over the ncand candidates ----
                # gxa row: [xA(32) auxA(32) xB(32) auxB(32) xsqA xsqB pad2]
                base = gxa[:, :, 0:4 * D].rearrange("p c (h v) -> p c h v", h=2)
                gx = base[:, :, :, 0:D]
                ga = base[:, :, :, D:2 * D]
                xsq = gxa[:, :, 4 * D:4 * D + 2]          # [128, npair_c, 2]
                # dots[q, c, h] = q . x
                pr = gp.tile([128, npair_c, 2, D], F32, tag="pr")
                nc.vector.tensor_tensor(
                    out=pr[:], in0=gx,
                    in1=qft[:].unsqueeze(1).unsqueeze(1)
                              .to_broadcast([128, npair_c, 2, D]),
                    op=mybir.AluOpType.mult)
                dots = sp.tile([128, npair_c, 2], F32, tag="dots")
                nc.vector.tensor_reduce(out=dots[:], in_=pr[:],
                                        axis=mybir.AxisListType.X,
                                        op=mybir.AluOpType.add)
                # neg2 = 2*dots - xsq  (dsq = qsq - neg2)
                neg2 = sp.tile([128, ncand], F32, tag="neg2")
                nc.vector.scalar_tensor_tensor(
                    out=neg2[:].rearrange("p (c h) -> p c h", h=2),
                    in0=dots[:], scalar=2.0, in1=xsq,
                    op0=mybir.AluOpType.mult, op1=mybir.AluOpType.subtract)
                t8 = sp.tile([128, 8], F32, tag="t8")
                nc.vector.max(out=t8[:], in_=neg2[:])
                mask = sp.tile([128, ncand], F32, tag="mask")
                nc.vector.tensor_scalar(mask[:], neg2[:], t8[:, 4:5], None,
                                        op0=mybir.AluOpType.is_ge)
                dsq = sp.tile([128, ncand], F32, tag="dsq")
                nc.vector.tensor_scalar(dsq[:], neg2[:], -1.0, qs[:, 0:1],
                                        op0=mybir.AluOpType.mult,
                                        op1=mybir.AluOpType.add)
                nc.vector.tensor_scalar_max(dsq[:], dsq[:], 0.0)
                dist = sp.tile([128, ncand], F32, tag="dist")
                nc.scalar.sqrt(out=dist[:], in_=dsq[:])
                nc.vector.tensor_scalar_add(dist[:], dist[:], EPS)
                rec = sp.tile([128, ncand], F32, tag="rec")
                nc.vector.reciprocal(out=rec[:], in_=dist[:])
                wgt = sp.tile([128, ncand], F32, tag="wgt")
                nc.vector.tensor_tensor(out=wgt[:], in0=rec[:], in1=mask[:],
                                        op=mybir.AluOpType.mult)
                wsum = sp.tile([128, 1], F32, tag="wsum")
                nc.vector.tensor_reduce(out=wsum[:], in_=wgt[:],
                                        axis=mybir.AxisListType.X,
                                        op=mybir.AluOpType.add)
                winv = sp.tile([128, 1], F32, tag="winv")
                nc.vector.reciprocal(out=winv[:], in_=wsum[:])

                # weighted sum of gathered aux rows
                prod = gp.tile([128, npair_c, 2, D], F32, tag="prod")
                nc.vector.tensor_tensor(
                    out=prod[:], in0=ga,
                    in1=wgt[:].rearrange("p (c h) -> p c h", h=2).unsqueeze(-1)
                              .to_broadcast([128, npair_c, 2, D]),
                    op=mybir.AluOpType.mult)
                acc = sp.tile([128, D], F32, tag="accr")
                nc.vector.tensor_reduce(
                    out=acc[:],
                    in_=prod[:].rearrange("p i h a -> p a (i h)"),
                    axis=mybir.AxisListType.X, op=mybir.AluOpType.add)
                outt = sp.tile([128, D], F32, tag="outt")
                nc.vector.tensor_scalar(outt[:], acc[:], winv[:, 0:1], None,
                                        op0=mybir.AluOpType.mult)
                nc.sync.dma_start(out=out[t * 128:(t + 1) * 128, :], in_=outt[:])

    split_multi_waits(nc)
    return nc


def split_multi_waits(nc):
    """The walrus build in this container supports a single sync-wait per
    instruction; Tile's tail drain carries one wait per live proc.  Split
    any multi-wait instruction into single-wait NoOps ahead of it."""
    for f in nc.m.functions:
        for blk in f.blocks:
            newinsts = []
            for ins in blk.instructions:
                si = ins.sync_info
                if si is not None and si.on_wait and len(si.on_wait) > 1:
                    waits = list(si.on_wait)
                    for k, w in enumerate(waits[:-1]):
                        nop = mybir.InstNoOp(name=f"{ins.name}-ws{k}", ins=[],
                                             outs=[])
                        nop.engine = ins.engine
                        nop.sync_info = mybir.SyncInfo(on_wait=[w], on_update=[])
                        newinsts.append(nop)
                    ins.sync_info = mybir.SyncInfo(on_wait=[waits[-1]],
                                                   on_update=list(si.on_update))
                newinsts.append(ins)
            blk.instructions = newinsts


def make_in_maps(embedding_features, db_embedding, auxiliary_features):
    q = np.ascontiguousarray(np.asarray(embedding_features, dtype=np.float32))
    db = np.ascontiguousarray(np.asarray(db_embedding, dtype=np.float32))
    aux = np.ascontiguousarray(np.asarray(auxiliary_features, dtype=np.float32))
    ndb = db.shape[0]
    nq_core = q.shape[0] // N_CORES
    bf = ml_dtypes.bfloat16
    bias = -0.5 * (db * db).sum(1)                      # exact f32
    b_hi = bias.astype(bf).astype(np.float32)
    b_lo = (bias - b_hi).astype(bf)
    dbT_aug = np.ascontiguousarray(np.concatenate(
        [db.T.astype(bf), b_hi.astype(bf)[None, :], b_lo[None, :]], axis=0,
        dtype=bf))
    # paired table: fold slot (w, p, u) covers db rows j1 = w*win + 2p*S + u
    # and j2 = j1 + S
    win = min(WINDOW, ndb)
    n_win = ndb // win
    n_pair = win // (2 * SUPER)
    idx = np.arange(ndb // 2)
    w_i = idx // (win // 2)
    rem = idx % (win // 2)
    p_i = rem // SUPER
    u_i = rem % SUPER
    j1 = w_i * win + 2 * p_i * SUPER + u_i
    j2 = j1 + SUPER
    dbsq = (db * db).sum(1)
    pair_table = np.zeros((ndb // 2, PV), np.float32)
    pair_table[:, 0:D] = db[j1]
    pair_table[:, D:2 * D] = aux[j1]
    pair_table[:, 2 * D:3 * D] = db[j2]
    pair_table[:, 3 * D:4 * D] = aux[j2]
    pair_table[:, 4 * D] = dbsq[j1]
    pair_table[:, 4 * D + 1] = dbsq[j2]
    pair_table = np.ascontiguousarray(pair_table)
    in_maps = []
    for c in range(N_CORES):
        qs = q[c * nq_core:(c + 1) * nq_core]
        qT_aug = np.ascontiguousarray(np.concatenate(
            [qs.T.astype(bf), np.ones((2, nq_core), bf)], axis=0, dtype=bf))
        qsq = np.ascontiguousarray((qs * qs).sum(1).reshape(nq_core, 1)
                                   ).astype(np.float32)
        in_maps.append({"qT_aug": qT_aug, "qf": qs, "qsq": qsq,
                        "dbT_aug": dbT_aug, "pair_table": pair_table})
    return in_maps


_NC_CACHE = {}


def get_nc(nq_core=NQ_CORE, ndb=NDB):
    key = (nq_core, ndb)
    if key not in _NC_CACHE:
        _NC_CACHE[key] = build_nc(nq_core, ndb)
    return _NC_CACHE[key]


def kernel(embedding_features, db_embedding, auxiliary_features):
    in_maps = make_in_maps(embedding_features, db_embedding, auxiliary_features)
    nc = get_nc()
    res = run_bass_kernel_spmd(nc, in_maps, list(range(N_CORES)))
    return np.concatenate([res.results[c]["out"] for c in range(N_CORES)],
                          axis=0).astype(np.float32)


# revision 14
# speedup vs baseline: 1.1256x; 1.1256x over previous
"""Trainium2 Bass kernel for nn_BaseEmbedder (retrieval_knn).

For each of 4096 query embeddings: find the 5 nearest of 65536 db embeddings
(Euclidean) and produce the inverse-distance-weighted sum of their auxiliary
features.  SPMD on 8 NeuronCores: queries sharded 512/core, db+aux replicated.

Per core (512 queries = 4 q-tiles of 128 partitions):
  - Scan (bf16): negS[q,j] = q.x_j - 0.5|x_j|^2 via K=34 augmented bf16
    matmuls (rows 32/33 carry the -0.5|x|^2 bias split hi/lo).  Two matmul
    streams run concurrently on PE row-groups 0 and 64 (even/odd 1024-col
    supers).
  - Pair-fold: DVE tensor_tensor(max) folds each super pair (PSUM A operand,
    SBUF copy of B) into zfold[u] = max(y_even[u], y_odd[u]) - 8192 values
    per 16384-column window.
  - Candidates: per window, max8 over zfold gives the top-8 folded values;
    max_index recovers their fold slots (needles are window-local, exact f32
    match).  Each slot maps to TWO db rows (the fold pair); both are
    gathered, so no parity disambiguation is needed.
  - Exact refinement (f32): a host-prepared paired table
    row[w*8192+p*1024+u] = [x_j1, aux_j1, x_j2, aux_j2, |x_j1|^2, |x_j2|^2]
    is gathered per winning slot via indirect DMA (one row per candidate
    pair per query).  Exact distances for all 64 candidates are recomputed
    on-chip; top-5 by threshold; weights 1/(d+eps) normalized; weighted sum.

The bf16 scan only nominates candidates; all selection/weight math is exact
f32, so the result matches the f32 reference to ~1e-6.
"""

import numpy as np
import ml_dtypes

from concourse import bass, mybir
from concourse.tile import TileContext
from concourse.bass_utils import run_bass_kernel_spmd

F32 = mybir.dt.float32
BF16 = mybir.dt.bfloat16
U32 = mybir.dt.uint32
I32 = mybir.dt.int32

N_CORES = 8
NQ = 4096
NDB = 65536
D = 32
DAUG = 34   # 32 dims + bias row + bias-residual row (bf16 split)
K = 5
EPS = 1e-6

NQ_CORE = NQ // N_CORES          # 512
CHUNK = 512                      # db columns per matmul (one PSUM bank)
SUPER = 1024                     # db columns per PSUM tile / fold operand
WINDOW = 32768                   # raw db columns per max8 + max_index window
RG_B = 64                        # partition base of the second PE row-group
PV = 132                         # paired-table row: xA,auxA,xB,auxB,xsqA,xsqB,pad


def build_nc(nq_core=NQ_CORE, ndb=NDB):
    n_qt = nq_core // 128
    win = min(WINDOW, ndb)
    n_win = ndb // win
    n_pair = win // (2 * SUPER)          # fold pairs per window (8)
    fold_w = win // 2                    # folded columns per window (8192)
    npair_c = 8 * n_win                  # candidate pairs per query
    ncand = 2 * npair_c                  # candidates per query

    nc = bass.Bass()
    qT = nc.declare_dram_parameter("qT_aug", [DAUG, nq_core], BF16, isOutput=False)
    qf = nc.declare_dram_parameter("qf", [nq_core, D], F32, isOutput=False)
    qsq = nc.declare_dram_parameter("qsq", [nq_core, 1], F32, isOutput=False)
    dbT = nc.declare_dram_parameter("dbT_aug", [DAUG, ndb], BF16, isOutput=False)
    pairt = nc.declare_dram_parameter("pair_table", [ndb // 2, PV], F32,
                                      isOutput=False)
    out = nc.declare_dram_parameter("out", [nq_core, D], F32, isOutput=True)

    with TileContext(nc) as tc:
        with (
            tc.tile_pool(name="zf", bufs=1) as zfp,
            tc.tile_pool(name="db", bufs=3) as dbp,
            tc.tile_pool(name="sbA", bufs=5) as sap,
            tc.tile_pool(name="sbB", bufs=5) as sbp,
            tc.tile_pool(name="psA", bufs=2, space="PSUM") as pspA,
            tc.tile_pool(name="psB", bufs=2, space="PSUM") as pspB,
            tc.tile_pool(name="sm", bufs=2) as sp,
            tc.tile_pool(name="g", bufs=2) as gp,
        ):
            for t in range(n_qt):
                # queries live on row groups 0 and RG_B so two matmul streams
                # run concurrently on the PE array
                qt = sp.tile([128, 128], BF16, tag="qt")
                nc.sync.dma_start(out=qt[0:DAUG, :],
                                  in_=qT[:, t * 128:(t + 1) * 128])
                nc.sync.dma_start(out=qt[RG_B:RG_B + DAUG, :],
                                  in_=qT[:, t * 128:(t + 1) * 128])
                qs = sp.tile([128, 1], F32, tag="qs")
                nc.sync.dma_start(out=qs[:], in_=qsq[t * 128:(t + 1) * 128, :])
                qft = sp.tile([128, D], F32, tag="qft")
                nc.sync.dma_start(out=qft[:], in_=qf[t * 128:(t + 1) * 128, :])

                candv = sp.tile([128, npair_c], F32, tag="candv")
                gxa = gp.tile([128, npair_c, PV], F32, tag="gxa")

                for w in range(n_win):
                    zfold = zfp.tile([128, fold_w], F32)
                    for p in range(n_pair):
                        offa = w * win + (2 * p) * SUPER
                        rhs = dbp.tile([128, SUPER], BF16)
                        nc.sync.dma_start(out=rhs[0:DAUG, :],
                                          in_=dbT[:, offa:offa + SUPER])
                        nc.sync.dma_start(
                            out=rhs[RG_B:RG_B + DAUG, :],
                            in_=dbT[:, offa + SUPER:offa + 2 * SUPER])
                        psA = pspA.tile([128, SUPER], F32, tag="psA")
                        psB = pspB.tile([128, SUPER], F32, tag="psB")
                        for m in range(SUPER // CHUNK):
                            sl = slice(m * CHUNK, (m + 1) * CHUNK)
                            nc.tensor.matmul(out=psA[:, sl],
                                             lhsT=qt[0:DAUG, :],
                                             rhs=rhs[0:DAUG, sl],
                                             start=True, stop=True,
                                             tile_position=(0, 0))
                            nc.tensor.matmul(out=psB[:, sl],
                                             lhsT=qt[RG_B:RG_B + DAUG, :],
                                             rhs=rhs[RG_B:RG_B + DAUG, sl],
                                             start=True, stop=True,
                                             tile_position=(RG_B, 0))
                        sbA = sap.tile([128, SUPER], F32)
                        nc.scalar.copy(out=sbA[:], in_=psA[:])
                        sbB = sbp.tile([128, SUPER], F32)
                        nc.scalar.copy(out=sbB[:], in_=psB[:])
                        nc.vector.tensor_tensor(
                            out=zfold[:, p * SUPER:(p + 1) * SUPER],
                            in0=sbA[:], in1=sbB[:], op=mybir.AluOpType.max)
                    # window top-8 folded values + their fold slots
                    w8 = candv[:, w * 8:(w + 1) * 8]
                    nc.vector.max(out=w8, in_=zfold[:])
                    pos = sp.tile([128, 8], U32, tag="pos")
                    nc.vector.max_index(out=pos[:], in_max=w8,
                                        in_values=zfold[:])
                    # paired-table row = slot + w*fold_w; gather immediately
                    ji = sp.tile([128, 8], I32, tag="ji")
                    nc.vector.tensor_scalar_add(ji[:], pos[:],
                                                float(w * fold_w))
                    for i in range(8):
                        nc.gpsimd.indirect_dma_start(
                            out=gxa[:, w * 8 + i, :], out_offset=None,
                            in_=pairt[:],
                            in_offset=bass.IndirectOffsetOnAxis(
                                ap=ji[:, i:i + 1], axis=0))

                # ---- exact f32 refinement over the ncand candidates ----
                # gxa row: [xA(32) auxA(32) xB(32) auxB(32) xsqA xsqB pad2]
                base = gxa[:, :, 0:4 * D].rearrange("p c (h v) -> p c h v", h=2)
                gx = base[:, :, :, 0:D]
                ga = base[:, :, :, D:2 * D]
                xsq = gxa[:, :, 4 * D:4 * D + 2]          # [128, npair_c, 2]
                # dots[q, c, h] = q . x
                pr = gp.tile([128, npair_c, 2, D], F32, tag="pr")
                nc.vector.tensor_tensor(
                    out=pr[:], in0=gx,
                    in1=qft[:].unsqueeze(1).unsqueeze(1)
                              .to_broadcast([128, npair_c, 2, D]),
                    op=mybir.AluOpType.mult)
                dots = sp.tile([128, npair_c, 2], F32, tag="dots")
                nc.vector.tensor_reduce(out=dots[:], in_=pr[:],
                                        axis=mybir.AxisListType.X,
                                        op=mybir.AluOpType.add)
                # neg2 = 2*dots - xsq  (dsq = qsq - neg2)
                neg2 = sp.tile([128, ncand], F32, tag="neg2")
                nc.vector.scalar_tensor_tensor(
                    out=neg2[:].rearrange("p (c h) -> p c h", h=2),
                    in0=dots[:], scalar=2.0, in1=xsq,
                    op0=mybir.AluOpType.mult, op1=mybir.AluOpType.subtract)
                t8 = sp.tile([128, 8], F32, tag="t8")
                nc.vector.max(out=t8[:], in_=neg2[:])
                mask = sp.tile([128, ncand], F32, tag="mask")
                nc.vector.tensor_scalar(mask[:], neg2[:], t8[:, 4:5], None,
                                        op0=mybir.AluOpType.is_ge)
                dsq = sp.tile([128, ncand], F32, tag="dsq")
                nc.vector.tensor_scalar(dsq[:], neg2[:], -1.0, qs[:, 0:1],
                                        op0=mybir.AluOpType.mult,
                                        op1=mybir.AluOpType.add)
                nc.vector.tensor_scalar_max(dsq[:], dsq[:], 0.0)
                dist = sp.tile([128, ncand], F32, tag="dist")
                nc.scalar.sqrt(out=dist[:], in_=dsq[:])
                nc.vector.tensor_scalar_add(dist[:], dist[:], EPS)
                rec = sp.tile([128, ncand], F32, tag="rec")
                nc.vector.reciprocal(out=rec[:], in_=dist[:])
                wgt = sp.tile([128, ncand], F32, tag="wgt")
                nc.vector.tensor_tensor(out=wgt[:], in0=rec[:], in1=mask[:],
                                        op=mybir.AluOpType.mult)
                wsum = sp.tile([128, 1], F32, tag="wsum")
                nc.vector.tensor_reduce(out=wsum[:], in_=wgt[:],
                                        axis=mybir.AxisListType.X,
                                        op=mybir.AluOpType.add)
                winv = sp.tile([128, 1], F32, tag="winv")
                nc.vector.reciprocal(out=winv[:], in_=wsum[:])

                # weighted sum of gathered aux rows
                prod = gp.tile([128, npair_c, 2, D], F32, tag="prod")
                nc.vector.tensor_tensor(
                    out=prod[:], in0=ga,
                    in1=wgt[:].rearrange("p (c h) -> p c h", h=2).unsqueeze(-1)
                              .to_broadcast([128, npair_c, 2, D]),
                    op=mybir.AluOpType.mult)
                acc = sp.tile([128, D], F32, tag="accr")
                nc.vector.tensor_reduce(
                    out=acc[:],
                    in_=prod[:].rearrange("p i h a -> p a (i h)"),
                    axis=mybir.AxisListType.X, op=mybir.AluOpType.add)
                outt = sp.tile([128, D], F32, tag="outt")
                nc.vector.tensor_scalar(outt[:], acc[:], winv[:, 0:1], None,
                                        op0=mybir.AluOpType.mult)
                nc.sync.dma_start(out=out[t * 128:(t + 1) * 128, :], in_=outt[:])

    split_multi_waits(nc)
    return nc


def split_multi_waits(nc):
    """The walrus build in this container supports a single sync-wait per
    instruction; Tile's tail drain carries one wait per live proc.  Split
    any multi-wait instruction into single-wait NoOps ahead of it."""
    for f in nc.m.functions:
        for blk in f.blocks:
            newinsts = []
            for ins in blk.instructions:
                si = ins.sync_info
                if si is not None and si.on_wait and len(si.on_wait) > 1:
                    waits = list(si.on_wait)
                    for k, w in enumerate(waits[:-1]):
                        nop = mybir.InstNoOp(name=f"{ins.name}-ws{k}", ins=[],
                                             outs=[])
                        nop.engine = ins.engine
                        nop.sync_info = mybir.SyncInfo(on_wait=[w], on_update=[])
                        newinsts.append(nop)
                    ins.sync_info = mybir.SyncInfo(on_wait=[waits[-1]],
                                                   on_update=list(si.on_update))
                newinsts.append(ins)
            blk.instructions = newinsts


def make_in_maps(embedding_features, db_embedding, auxiliary_features):
    q = np.ascontiguousarray(np.asarray(embedding_features, dtype=np.float32))
    db = np.ascontiguousarray(np.asarray(db_embedding, dtype=np.float32))
    aux = np.ascontiguousarray(np.asarray(auxiliary_features, dtype=np.float32))
    ndb = db.shape[0]
    nq_core = q.shape[0] // N_CORES
    bf = ml_dtypes.bfloat16
    bias = -0.5 * (db * db).sum(1)                      # exact f32
    b_hi = bias.astype(bf).astype(np.float32)
    b_lo = (bias - b_hi).astype(bf)
    dbT_aug = np.ascontiguousarray(np.concatenate(
        [db.T.astype(bf), b_hi.astype(bf)[None, :], b_lo[None, :]], axis=0,
        dtype=bf))
    # paired table: fold slot (w, p, u) covers db rows j1 = w*win + 2p*S + u
    # and j2 = j1 + S
    win = min(WINDOW, ndb)
    n_win = ndb // win
    n_pair = win // (2 * SUPER)
    idx = np.arange(ndb // 2)
    w_i = idx // (win // 2)
    rem = idx % (win // 2)
    p_i = rem // SUPER
    u_i = rem % SUPER
    j1 = w_i * win + 2 * p_i * SUPER + u_i
    j2 = j1 + SUPER
    dbsq = (db * db).sum(1)
    pair_table = np.zeros((ndb // 2, PV), np.float32)
    pair_table[:, 0:D] = db[j1]
    pair_table[:, D:2 * D] = aux[j1]
    pair_table[:, 2 * D:3 * D] = db[j2]
    pair_table[:, 3 * D:4 * D] = aux[j2]
    pair_table[:, 4 * D] = dbsq[j1]
    pair_table[:, 4 * D + 1] = dbsq[j2]
    pair_table = np.ascontiguousarray(pair_table)
    in_maps = []
    for c in range(N_CORES):
        qs = q[c * nq_core:(c + 1) * nq_core]
        qT_aug = np.ascontiguousarray(np.concatenate(
            [qs.T.astype(bf), np.ones((2, nq_core), bf)], axis=0, dtype=bf))
        qsq = np.ascontiguousarray((qs * qs).sum(1).reshape(nq_core, 1)
                                   ).astype(np.float32)
        in_maps.append({"qT_aug": qT_aug, "qf": qs, "qsq": qsq,
                        "dbT_aug": dbT_aug, "pair_table": pair_table})
    return in_maps


_NC_CACHE = {}


def get_nc(nq_core=NQ_CORE, ndb=NDB):
    key = (nq_core, ndb)
    if key not in _NC_CACHE:
        _NC_CACHE[key] = build_nc(nq_core, ndb)
    return _NC_CACHE[key]


def kernel(embedding_features, db_embedding, auxiliary_features):
    in_maps = make_in_maps(embedding_features, db_embedding, auxiliary_features)
    nc = get_nc()
    res = run_bass_kernel_spmd(nc, in_maps, list(range(N_CORES)))
    return np.concatenate([res.results[c]["out"] for c in range(N_CORES)],
                          axis=0).astype(np.float32)


# revision 15
# speedup vs baseline: 1.2253x; 1.0886x over previous
"""Trainium2 Bass kernel for nn_BaseEmbedder (retrieval_knn).

For each of 4096 query embeddings: find the 5 nearest of 65536 db embeddings
(Euclidean) and produce the inverse-distance-weighted sum of their auxiliary
features.  SPMD on 8 NeuronCores: queries sharded 512/core, db+aux replicated.

Per core (512 queries = 4 q-tiles of 128 partitions):
  - Scan (bf16): negS[q,j] = q.x_j - 0.5|x_j|^2 via K=34 augmented bf16
    matmuls (rows 32/33 carry the -0.5|x|^2 bias split hi/lo).  Two matmul
    streams run concurrently on PE row-groups 0 and 64 (even/odd 1024-col
    supers).
  - Pair-fold: DVE tensor_tensor(max) folds each super pair (PSUM A operand,
    SBUF copy of B) into zfold[u] = max(y_even[u], y_odd[u]) - 8192 values
    per 16384-column window.
  - Candidates: per window, max8 over zfold gives the top-8 folded values;
    max_index recovers their fold slots (needles are window-local, exact f32
    match).  Each slot maps to TWO db rows (the fold pair); both are
    gathered, so no parity disambiguation is needed.
  - Exact refinement (f32): a host-prepared paired table
    row[w*8192+p*1024+u] = [x_j1, aux_j1, x_j2, aux_j2, |x_j1|^2, |x_j2|^2]
    is gathered per winning slot via indirect DMA (one row per candidate
    pair per query).  Exact distances for all 64 candidates are recomputed
    on-chip; top-5 by threshold; weights 1/(d+eps) normalized; weighted sum.

The bf16 scan only nominates candidates; all selection/weight math is exact
f32, so the result matches the f32 reference to ~1e-6.
"""

import numpy as np
import ml_dtypes

from concourse import bass, mybir
from concourse.tile import TileContext
from concourse.bass_utils import run_bass_kernel_spmd

F32 = mybir.dt.float32
BF16 = mybir.dt.bfloat16
U32 = mybir.dt.uint32
I32 = mybir.dt.int32

N_CORES = 8
NQ = 4096
NDB = 65536
D = 32
DAUG = 34   # 32 dims + bias row + bias-residual row (bf16 split)
K = 5
EPS = 1e-6

NQ_CORE = NQ // N_CORES          # 512
CHUNK = 512                      # db columns per matmul (one PSUM bank)
SUPER = 1024                     # db columns per PSUM tile / fold operand
WINDOW = 32768                   # raw db columns per max8 + max_index window
RG_B = 64                        # partition base of the second PE row-group
PV = 132                         # paired-table row: xA,auxA,xB,auxB,xsqA,xsqB,pad


def build_nc(nq_core=NQ_CORE, ndb=NDB):
    n_qt = nq_core // 128
    win = min(WINDOW, ndb)
    n_win = ndb // win
    n_pair = win // (2 * SUPER)          # fold pairs per window (8)
    fold_w = win // 2                    # folded columns per window (8192)
    npair_c = 8 * n_win                  # candidate pairs per query
    ncand = 2 * npair_c                  # candidates per query

    nc = bass.Bass()
    qT = nc.declare_dram_parameter("qT_aug", [DAUG, nq_core], BF16, isOutput=False)
    qf = nc.declare_dram_parameter("qf", [nq_core, D], F32, isOutput=False)
    qsq = nc.declare_dram_parameter("qsq", [nq_core, 1], F32, isOutput=False)
    dbT = nc.declare_dram_parameter("dbT_aug", [DAUG, ndb], BF16, isOutput=False)
    pairt = nc.declare_dram_parameter("pair_table", [ndb // 2, PV], F32,
                                      isOutput=False)
    out = nc.declare_dram_parameter("out", [nq_core, D], F32, isOutput=True)

    with TileContext(nc) as tc:
        with (
            tc.tile_pool(name="zf", bufs=1) as zfp,
            tc.tile_pool(name="db", bufs=4) as dbp,
            tc.tile_pool(name="sbA", bufs=8) as sap,
            tc.tile_pool(name="sbB", bufs=8) as sbp,
            tc.tile_pool(name="psA", bufs=2, space="PSUM") as pspA,
            tc.tile_pool(name="psB", bufs=2, space="PSUM") as pspB,
            tc.tile_pool(name="sm", bufs=2) as sp,
            tc.tile_pool(name="g", bufs=2) as gp,
        ):
            for t in range(n_qt):
                # queries live on row groups 0 and RG_B so two matmul streams
                # run concurrently on the PE array
                qt = sp.tile([128, 128], BF16, tag="qt")
                nc.sync.dma_start(out=qt[0:DAUG, :],
                                  in_=qT[:, t * 128:(t + 1) * 128])
                nc.sync.dma_start(out=qt[RG_B:RG_B + DAUG, :],
                                  in_=qT[:, t * 128:(t + 1) * 128])
                qs = sp.tile([128, 1], F32, tag="qs")
                nc.sync.dma_start(out=qs[:], in_=qsq[t * 128:(t + 1) * 128, :])
                qft = sp.tile([128, D], F32, tag="qft")
                nc.sync.dma_start(out=qft[:], in_=qf[t * 128:(t + 1) * 128, :])

                candv = sp.tile([128, npair_c], F32, tag="candv")
                gxa = gp.tile([128, npair_c, PV], F32, tag="gxa")

                for w in range(n_win):
                    zfold = zfp.tile([128, fold_w], F32)
                    for p in range(n_pair):
                        offa = w * win + (2 * p) * SUPER
                        rhs = dbp.tile([128, SUPER], BF16)
                        nc.sync.dma_start(out=rhs[0:DAUG, :],
                                          in_=dbT[:, offa:offa + SUPER])
                        nc.sync.dma_start(
                            out=rhs[RG_B:RG_B + DAUG, :],
                            in_=dbT[:, offa + SUPER:offa + 2 * SUPER])
                        psA = pspA.tile([128, SUPER], F32, tag="psA")
                        psB = pspB.tile([128, SUPER], F32, tag="psB")
                        for m in range(SUPER // CHUNK):
                            sl = slice(m * CHUNK, (m + 1) * CHUNK)
                            nc.tensor.matmul(out=psA[:, sl],
                                             lhsT=qt[0:DAUG, :],
                                             rhs=rhs[0:DAUG, sl],
                                             start=True, stop=True,
                                             tile_position=(0, 0))
                            nc.tensor.matmul(out=psB[:, sl],
                                             lhsT=qt[RG_B:RG_B + DAUG, :],
                                             rhs=rhs[RG_B:RG_B + DAUG, sl],
                                             start=True, stop=True,
                                             tile_position=(RG_B, 0))
                        sbA = sap.tile([128, SUPER], F32)
                        nc.scalar.copy(out=sbA[:], in_=psA[:])
                        sbB = sbp.tile([128, SUPER], F32)
                        nc.scalar.copy(out=sbB[:], in_=psB[:])
                        nc.vector.tensor_tensor(
                            out=zfold[:, p * SUPER:(p + 1) * SUPER],
                            in0=sbA[:], in1=sbB[:], op=mybir.AluOpType.max)
                    # window top-8 folded values + their fold slots
                    w8 = candv[:, w * 8:(w + 1) * 8]
                    nc.vector.max(out=w8, in_=zfold[:])
                    pos = sp.tile([128, 8], U32, tag="pos")
                    nc.vector.max_index(out=pos[:], in_max=w8,
                                        in_values=zfold[:])
                    # paired-table row = slot + w*fold_w; gather immediately
                    ji = sp.tile([128, 8], I32, tag="ji")
                    nc.vector.tensor_scalar_add(ji[:], pos[:],
                                                float(w * fold_w))
                    for i in range(8):
                        nc.gpsimd.indirect_dma_start(
                            out=gxa[:, w * 8 + i, :], out_offset=None,
                            in_=pairt[:],
                            in_offset=bass.IndirectOffsetOnAxis(
                                ap=ji[:, i:i + 1], axis=0))

                # ---- exact f32 refinement over the ncand candidates ----
                # gxa row: [xA(32) auxA(32) xB(32) auxB(32) xsqA xsqB pad2]
                base = gxa[:, :, 0:4 * D].rearrange("p c (h v) -> p c h v", h=2)
                gx = base[:, :, :, 0:D]
                ga = base[:, :, :, D:2 * D]
                xsq = gxa[:, :, 4 * D:4 * D + 2]          # [128, npair_c, 2]
                # dots[q, c, h] = q . x
                pr = gp.tile([128, npair_c, 2, D], F32, tag="pr")
                nc.vector.tensor_tensor(
                    out=pr[:], in0=gx,
                    in1=qft[:].unsqueeze(1).unsqueeze(1)
                              .to_broadcast([128, npair_c, 2, D]),
                    op=mybir.AluOpType.mult)
                dots = sp.tile([128, npair_c, 2], F32, tag="dots")
                nc.vector.tensor_reduce(out=dots[:], in_=pr[:],
                                        axis=mybir.AxisListType.X,
                                        op=mybir.AluOpType.add)
                # neg2 = 2*dots - xsq  (dsq = qsq - neg2)
                neg2 = sp.tile([128, ncand], F32, tag="neg2")
                nc.vector.scalar_tensor_tensor(
                    out=neg2[:].rearrange("p (c h) -> p c h", h=2),
                    in0=dots[:], scalar=2.0, in1=xsq,
                    op0=mybir.AluOpType.mult, op1=mybir.AluOpType.subtract)
                t8 = sp.tile([128, 8], F32, tag="t8")
                nc.vector.max(out=t8[:], in_=neg2[:])
                mask = sp.tile([128, ncand], F32, tag="mask")
                nc.vector.tensor_scalar(mask[:], neg2[:], t8[:, 4:5], None,
                                        op0=mybir.AluOpType.is_ge)
                dsq = sp.tile([128, ncand], F32, tag="dsq")
                nc.vector.tensor_scalar(dsq[:], neg2[:], -1.0, qs[:, 0:1],
                                        op0=mybir.AluOpType.mult,
                                        op1=mybir.AluOpType.add)
                nc.vector.tensor_scalar_max(dsq[:], dsq[:], 0.0)
                dist = sp.tile([128, ncand], F32, tag="dist")
                nc.scalar.sqrt(out=dist[:], in_=dsq[:])
                nc.vector.tensor_scalar_add(dist[:], dist[:], EPS)
                rec = sp.tile([128, ncand], F32, tag="rec")
                nc.vector.reciprocal(out=rec[:], in_=dist[:])
                wgt = sp.tile([128, ncand], F32, tag="wgt")
                nc.vector.tensor_tensor(out=wgt[:], in0=rec[:], in1=mask[:],
                                        op=mybir.AluOpType.mult)
                wsum = sp.tile([128, 1], F32, tag="wsum")
                nc.vector.tensor_reduce(out=wsum[:], in_=wgt[:],
                                        axis=mybir.AxisListType.X,
                                        op=mybir.AluOpType.add)
                winv = sp.tile([128, 1], F32, tag="winv")
                nc.vector.reciprocal(out=winv[:], in_=wsum[:])

                # weighted sum of gathered aux rows
                prod = gp.tile([128, npair_c, 2, D], F32, tag="prod")
                nc.vector.tensor_tensor(
                    out=prod[:], in0=ga,
                    in1=wgt[:].rearrange("p (c h) -> p c h", h=2).unsqueeze(-1)
                              .to_broadcast([128, npair_c, 2, D]),
                    op=mybir.AluOpType.mult)
                acc = sp.tile([128, D], F32, tag="accr")
                nc.vector.tensor_reduce(
                    out=acc[:],
                    in_=prod[:].rearrange("p i h a -> p a (i h)"),
                    axis=mybir.AxisListType.X, op=mybir.AluOpType.add)
                outt = sp.tile([128, D], F32, tag="outt")
                nc.vector.tensor_scalar(outt[:], acc[:], winv[:, 0:1], None,
                                        op0=mybir.AluOpType.mult)
                nc.sync.dma_start(out=out[t * 128:(t + 1) * 128, :], in_=outt[:])

    split_multi_waits(nc)
    return nc


def split_multi_waits(nc):
    """The walrus build in this container supports a single sync-wait per
    instruction; Tile's tail drain carries one wait per live proc.  Split
    any multi-wait instruction into single-wait NoOps ahead of it."""
    for f in nc.m.functions:
        for blk in f.blocks:
            newinsts = []
            for ins in blk.instructions:
                si = ins.sync_info
                if si is not None and si.on_wait and len(si.on_wait) > 1:
                    waits = list(si.on_wait)
                    for k, w in enumerate(waits[:-1]):
                        nop = mybir.InstNoOp(name=f"{ins.name}-ws{k}", ins=[],
                                             outs=[])
                        nop.engine = ins.engine
                        nop.sync_info = mybir.SyncInfo(on_wait=[w], on_update=[])
                        newinsts.append(nop)
                    ins.sync_info = mybir.SyncInfo(on_wait=[waits[-1]],
                                                   on_update=list(si.on_update))
                newinsts.append(ins)
            blk.instructions = newinsts


def make_in_maps(embedding_features, db_embedding, auxiliary_features):
    q = np.ascontiguousarray(np.asarray(embedding_features, dtype=np.float32))
    db = np.ascontiguousarray(np.asarray(db_embedding, dtype=np.float32))
    aux = np.ascontiguousarray(np.asarray(auxiliary_features, dtype=np.float32))
    ndb = db.shape[0]
    nq_core = q.shape[0] // N_CORES
    bf = ml_dtypes.bfloat16
    bias = -0.5 * (db * db).sum(1)                      # exact f32
    b_hi = bias.astype(bf).astype(np.float32)
    b_lo = (bias - b_hi).astype(bf)
    dbT_aug = np.ascontiguousarray(np.concatenate(
        [db.T.astype(bf), b_hi.astype(bf)[None, :], b_lo[None, :]], axis=0,
        dtype=bf))
    # paired table: fold slot (w, p, u) covers db rows j1 = w*win + 2p*S + u
    # and j2 = j1 + S
    win = min(WINDOW, ndb)
    n_win = ndb // win
    n_pair = win // (2 * SUPER)
    idx = np.arange(ndb // 2)
    w_i = idx // (win // 2)
    rem = idx % (win // 2)
    p_i = rem // SUPER
    u_i = rem % SUPER
    j1 = w_i * win + 2 * p_i * SUPER + u_i
    j2 = j1 + SUPER
    dbsq = (db * db).sum(1)
    pair_table = np.zeros((ndb // 2, PV), np.float32)
    pair_table[:, 0:D] = db[j1]
    pair_table[:, D:2 * D] = aux[j1]
    pair_table[:, 2 * D:3 * D] = db[j2]
    pair_table[:, 3 * D:4 * D] = aux[j2]
    pair_table[:, 4 * D] = dbsq[j1]
    pair_table[:, 4 * D + 1] = dbsq[j2]
    pair_table = np.ascontiguousarray(pair_table)
    in_maps = []
    for c in range(N_CORES):
        qs = q[c * nq_core:(c + 1) * nq_core]
        qT_aug = np.ascontiguousarray(np.concatenate(
            [qs.T.astype(bf), np.ones((2, nq_core), bf)], axis=0, dtype=bf))
        qsq = np.ascontiguousarray((qs * qs).sum(1).reshape(nq_core, 1)
                                   ).astype(np.float32)
        in_maps.append({"qT_aug": qT_aug, "qf": qs, "qsq": qsq,
                        "dbT_aug": dbT_aug, "pair_table": pair_table})
    return in_maps


_NC_CACHE = {}


def get_nc(nq_core=NQ_CORE, ndb=NDB):
    key = (nq_core, ndb)
    if key not in _NC_CACHE:
        _NC_CACHE[key] = build_nc(nq_core, ndb)
    return _NC_CACHE[key]


def kernel(embedding_features, db_embedding, auxiliary_features):
    in_maps = make_in_maps(embedding_features, db_embedding, auxiliary_features)
    nc = get_nc()
    res = run_bass_kernel_spmd(nc, in_maps, list(range(N_CORES)))
    return np.concatenate([res.results[c]["out"] for c in range(N_CORES)],
                          axis=0).astype(np.float32)


# revision 17
# speedup vs baseline: 1.3138x; 1.0722x over previous
"""Trainium2 Bass kernel for nn_BaseEmbedder (retrieval_knn).

For each of 4096 query embeddings: find the 5 nearest of 65536 db embeddings
(Euclidean) and produce the inverse-distance-weighted sum of their auxiliary
features.  SPMD on 8 NeuronCores: queries sharded 512/core, db+aux replicated.

Per core (512 queries = 4 q-tiles of 128 partitions):
  - Scan (bf16): negS[q,j] = q.x_j - 0.5|x_j|^2 via K=34 augmented bf16
    matmuls (rows 32/33 carry the -0.5|x|^2 bias split hi/lo).  Two matmul
    streams run concurrently on PE row-groups 0 and 64 (even/odd 1024-col
    supers).
  - Pair-fold: DVE tensor_tensor(max) folds each super pair (PSUM A operand,
    SBUF copy of B) into zfold[u] = max(y_even[u], y_odd[u]) - 8192 values
    per 16384-column window.
  - Candidates: per window, max8 over zfold gives the top-8 folded values;
    max_index recovers their fold slots (needles are window-local, exact f32
    match).  Each slot maps to TWO db rows (the fold pair); both are
    gathered, so no parity disambiguation is needed.
  - Exact refinement (f32): a host-prepared paired table
    row[w*8192+p*1024+u] = [x_j1, aux_j1, x_j2, aux_j2, |x_j1|^2, |x_j2|^2]
    is gathered per winning slot via indirect DMA (one row per candidate
    pair per query).  Exact distances for all 64 candidates are recomputed
    on-chip; top-5 by threshold; weights 1/(d+eps) normalized; weighted sum.

The bf16 scan only nominates candidates; all selection/weight math is exact
f32, so the result matches the f32 reference to ~1e-6.
"""

import numpy as np
import ml_dtypes

from concourse import bass, mybir
from concourse.tile import TileContext
from concourse.bass_utils import run_bass_kernel_spmd

F32 = mybir.dt.float32
BF16 = mybir.dt.bfloat16
U32 = mybir.dt.uint32
I32 = mybir.dt.int32

N_CORES = 8
NQ = 4096
NDB = 65536
D = 32
DAUG = 34   # 32 dims + bias row + bias-residual row (bf16 split)
K = 5
EPS = 1e-6

NQ_CORE = NQ // N_CORES          # 512
CHUNK = 512                      # db columns per matmul (one PSUM bank)
SUPER = 1024                     # db columns per PSUM tile / fold operand
RG_B = 64                        # partition base of the second PE row-group
PV = 260                         # paired row: 4x [x(32) aux(32)] + 4x |x|^2


def build_nc(nq_core=NQ_CORE, ndb=NDB):
    n_qt = nq_core // 128
    n_grp = ndb // (4 * SUPER)           # 4-super fold groups (16)
    fold_w = ndb // 4                    # folded columns (16384, one window)
    assert fold_w <= 16384
    ncand = 4 * 8                        # 8 needles x 4 rows per fold slot

    nc = bass.Bass()
    qT = nc.declare_dram_parameter("qT_aug", [DAUG, nq_core], BF16, isOutput=False)
    qf = nc.declare_dram_parameter("qf", [nq_core, D], F32, isOutput=False)
    qsq = nc.declare_dram_parameter("qsq", [nq_core, 1], F32, isOutput=False)
    dbT = nc.declare_dram_parameter("dbT_aug", [DAUG, ndb], BF16, isOutput=False)
    pairt = nc.declare_dram_parameter("pair_table", [ndb // 4, PV], F32,
                                      isOutput=False)
    out = nc.declare_dram_parameter("out", [nq_core, D], F32, isOutput=True)

    with TileContext(nc) as tc:
        with (
            tc.tile_pool(name="zf", bufs=1) as zfp,
            tc.tile_pool(name="db", bufs=4) as dbp,
            tc.tile_pool(name="sbA", bufs=6) as sap,
            tc.tile_pool(name="sbB", bufs=6) as sbp,
            tc.tile_pool(name="tf", bufs=2) as tfp,
            tc.tile_pool(name="psA", bufs=2, space="PSUM") as pspA,
            tc.tile_pool(name="psB", bufs=2, space="PSUM") as pspB,
            tc.tile_pool(name="sm", bufs=2) as sp,
            tc.tile_pool(name="g", bufs=1) as gp,
        ):
            for t in range(n_qt):
                # queries live on row groups 0 and RG_B so two matmul streams
                # run concurrently on the PE array
                qt = sp.tile([128, 128], BF16, tag="qt")
                nc.sync.dma_start(out=qt[0:DAUG, :],
                                  in_=qT[:, t * 128:(t + 1) * 128])
                nc.sync.dma_start(out=qt[RG_B:RG_B + DAUG, :],
                                  in_=qT[:, t * 128:(t + 1) * 128])
                qs = sp.tile([128, 1], F32, tag="qs")
                nc.sync.dma_start(out=qs[:], in_=qsq[t * 128:(t + 1) * 128, :])
                qft = sp.tile([128, D], F32, tag="qft")
                nc.sync.dma_start(out=qft[:], in_=qf[t * 128:(t + 1) * 128, :])

                candv = sp.tile([128, 8], F32, tag="candv")
                gxa = gp.tile([128, 8, PV], F32, tag="gxa")

                zfold = zfp.tile([128, fold_w], F32)
                for g in range(n_grp):
                    th = []
                    for h in range(2):
                        offa = (4 * g + 2 * h) * SUPER
                        rhs = dbp.tile([128, SUPER], BF16)
                        nc.sync.dma_start(out=rhs[0:DAUG, :],
                                          in_=dbT[:, offa:offa + SUPER])
                        nc.sync.dma_start(
                            out=rhs[RG_B:RG_B + DAUG, :],
                            in_=dbT[:, offa + SUPER:offa + 2 * SUPER])
                        psA = pspA.tile([128, SUPER], F32, tag="psA")
                        psB = pspB.tile([128, SUPER], F32, tag="psB")
                        for m in range(SUPER // CHUNK):
                            sl = slice(m * CHUNK, (m + 1) * CHUNK)
                            nc.tensor.matmul(out=psA[:, sl],
                                             lhsT=qt[0:DAUG, :],
                                             rhs=rhs[0:DAUG, sl],
                                             start=True, stop=True,
                                             tile_position=(0, 0))
                            nc.tensor.matmul(out=psB[:, sl],
                                             lhsT=qt[RG_B:RG_B + DAUG, :],
                                             rhs=rhs[RG_B:RG_B + DAUG, sl],
                                             start=True, stop=True,
                                             tile_position=(RG_B, 0))
                        sbA = sap.tile([128, SUPER], F32)
                        nc.scalar.copy(out=sbA[:], in_=psA[:])
                        sbB = sbp.tile([128, SUPER], F32)
                        nc.scalar.copy(out=sbB[:], in_=psB[:])
                        t1 = tfp.tile([128, SUPER], F32, tag=f"t{h}")
                        nc.vector.tensor_tensor(out=t1[:], in0=sbA[:],
                                                in1=sbB[:],
                                                op=mybir.AluOpType.max)
                        th.append(t1)
                    nc.vector.tensor_tensor(
                        out=zfold[:, g * SUPER:(g + 1) * SUPER],
                        in0=th[0][:], in1=th[1][:], op=mybir.AluOpType.max)
                # top-8 folded values + their fold slots (one window = all db)
                w8 = candv[:, 0:8]
                nc.vector.max(out=w8, in_=zfold[:])
                pos = sp.tile([128, 8], U32, tag="pos")
                nc.vector.max_index(out=pos[:], in_max=w8, in_values=zfold[:])
                ji = sp.tile([128, 8], I32, tag="ji")
                nc.vector.tensor_copy(ji[:], pos[:])
                for i in range(8):
                    nc.gpsimd.indirect_dma_start(
                        out=gxa[:, i, :], out_offset=None, in_=pairt[:],
                        in_offset=bass.IndirectOffsetOnAxis(
                            ap=ji[:, i:i + 1], axis=0))

                # ---- exact f32 refinement over the ncand candidates ----
                # gxa row: [xA(32) auxA(32) xB(32) auxB(32) xsqA xsqB pad2]
                base = gxa[:, :, 0:8 * D].rearrange("p c (h v) -> p c h v", h=4)
                gx = base[:, :, :, 0:D]
                ga = base[:, :, :, D:2 * D]
                xsq = gxa[:, :, 8 * D:8 * D + 4]          # [128, 8, 4]
                # dots[q, c, h] = q . x
                pr = gp.tile([128, 8, 4, D], F32, tag="pr")
                nc.vector.tensor_tensor(
                    out=pr[:], in0=gx,
                    in1=qft[:].unsqueeze(1).unsqueeze(1)
                              .to_broadcast([128, 8, 4, D]),
                    op=mybir.AluOpType.mult)
                dots = sp.tile([128, 8, 4], F32, tag="dots")
                nc.vector.tensor_reduce(out=dots[:], in_=pr[:],
                                        axis=mybir.AxisListType.X,
                                        op=mybir.AluOpType.add)
                # neg2 = 2*dots - xsq  (dsq = qsq - neg2)
                neg2 = sp.tile([128, ncand], F32, tag="neg2")
                nc.vector.scalar_tensor_tensor(
                    out=neg2[:].rearrange("p (c h) -> p c h", h=4),
                    in0=dots[:], scalar=2.0, in1=xsq,
                    op0=mybir.AluOpType.mult, op1=mybir.AluOpType.subtract)
                t8 = sp.tile([128, 8], F32, tag="t8")
                nc.vector.max(out=t8[:], in_=neg2[:])
                mask = sp.tile([128, ncand], F32, tag="mask")
                nc.vector.tensor_scalar(mask[:], neg2[:], t8[:, 4:5], None,
                                        op0=mybir.AluOpType.is_ge)
                dsq = sp.tile([128, ncand], F32, tag="dsq")
                nc.vector.tensor_scalar(dsq[:], neg2[:], -1.0, qs[:, 0:1],
                                        op0=mybir.AluOpType.mult,
                                        op1=mybir.AluOpType.add)
                nc.vector.tensor_scalar_max(dsq[:], dsq[:], 0.0)
                dist = sp.tile([128, ncand], F32, tag="dist")
                nc.scalar.sqrt(out=dist[:], in_=dsq[:])
                nc.vector.tensor_scalar_add(dist[:], dist[:], EPS)
                rec = sp.tile([128, ncand], F32, tag="rec")
                nc.vector.reciprocal(out=rec[:], in_=dist[:])
                wgt = sp.tile([128, ncand], F32, tag="wgt")
                nc.vector.tensor_tensor(out=wgt[:], in0=rec[:], in1=mask[:],
                                        op=mybir.AluOpType.mult)
                wsum = sp.tile([128, 1], F32, tag="wsum")
                nc.vector.tensor_reduce(out=wsum[:], in_=wgt[:],
                                        axis=mybir.AxisListType.X,
                                        op=mybir.AluOpType.add)
                winv = sp.tile([128, 1], F32, tag="winv")
                nc.vector.reciprocal(out=winv[:], in_=wsum[:])

                # weighted sum of gathered aux rows
                prod = gp.tile([128, 8, 4, D], F32, tag="prod")
                nc.vector.tensor_tensor(
                    out=prod[:], in0=ga,
                    in1=wgt[:].rearrange("p (c h) -> p c h", h=4).unsqueeze(-1)
                              .to_broadcast([128, 8, 4, D]),
                    op=mybir.AluOpType.mult)
                acc = sp.tile([128, D], F32, tag="accr")
                nc.vector.tensor_reduce(
                    out=acc[:],
                    in_=prod[:].rearrange("p i h a -> p a (i h)"),
                    axis=mybir.AxisListType.X, op=mybir.AluOpType.add)
                outt = sp.tile([128, D], F32, tag="outt")
                nc.vector.tensor_scalar(outt[:], acc[:], winv[:, 0:1], None,
                                        op0=mybir.AluOpType.mult)
                nc.sync.dma_start(out=out[t * 128:(t + 1) * 128, :], in_=outt[:])

    split_multi_waits(nc)
    return nc


def split_multi_waits(nc):
    """The walrus build in this container supports a single sync-wait per
    instruction; Tile's tail drain carries one wait per live proc.  Split
    any multi-wait instruction into single-wait NoOps ahead of it."""
    for f in nc.m.functions:
        for blk in f.blocks:
            newinsts = []
            for ins in blk.instructions:
                si = ins.sync_info
                if si is not None and si.on_wait and len(si.on_wait) > 1:
                    waits = list(si.on_wait)
                    for k, w in enumerate(waits[:-1]):
                        nop = mybir.InstNoOp(name=f"{ins.name}-ws{k}", ins=[],
                                             outs=[])
                        nop.engine = ins.engine
                        nop.sync_info = mybir.SyncInfo(on_wait=[w], on_update=[])
                        newinsts.append(nop)
                    ins.sync_info = mybir.SyncInfo(on_wait=[waits[-1]],
                                                   on_update=list(si.on_update))
                newinsts.append(ins)
            blk.instructions = newinsts


def make_in_maps(embedding_features, db_embedding, auxiliary_features):
    q = np.ascontiguousarray(np.asarray(embedding_features, dtype=np.float32))
    db = np.ascontiguousarray(np.asarray(db_embedding, dtype=np.float32))
    aux = np.ascontiguousarray(np.asarray(auxiliary_features, dtype=np.float32))
    ndb = db.shape[0]
    nq_core = q.shape[0] // N_CORES
    bf = ml_dtypes.bfloat16
    bias = -0.5 * (db * db).sum(1)                      # exact f32
    b_hi = bias.astype(bf).astype(np.float32)
    b_lo = (bias - b_hi).astype(bf)
    dbT_aug = np.ascontiguousarray(np.concatenate(
        [db.T.astype(bf), b_hi.astype(bf)[None, :], b_lo[None, :]], axis=0,
        dtype=bf))
    # paired table: fold slot s = g*S + u covers db rows (4g+m)*S + u, m=0..3
    idx = np.arange(ndb // 4)
    g_i = idx // SUPER
    u_i = idx % SUPER
    dbsq = (db * db).sum(1)
    pair_table = np.zeros((ndb // 4, PV), np.float32)
    for m in range(4):
        jm = (4 * g_i + m) * SUPER + u_i
        pair_table[:, 2 * m * D:(2 * m + 1) * D] = db[jm]
        pair_table[:, (2 * m + 1) * D:(2 * m + 2) * D] = aux[jm]
        pair_table[:, 8 * D + m] = dbsq[jm]
    pair_table = np.ascontiguousarray(pair_table)
    in_maps = []
    for c in range(N_CORES):
        qs = q[c * nq_core:(c + 1) * nq_core]
        qT_aug = np.ascontiguousarray(np.concatenate(
            [qs.T.astype(bf), np.ones((2, nq_core), bf)], axis=0, dtype=bf))
        qsq = np.ascontiguousarray((qs * qs).sum(1).reshape(nq_core, 1)
                                   ).astype(np.float32)
        in_maps.append({"qT_aug": qT_aug, "qf": qs, "qsq": qsq,
                        "dbT_aug": dbT_aug, "pair_table": pair_table})
    return in_maps


_NC_CACHE = {}


def get_nc(nq_core=NQ_CORE, ndb=NDB):
    key = (nq_core, ndb)
    if key not in _NC_CACHE:
        _NC_CACHE[key] = build_nc(nq_core, ndb)
    return _NC_CACHE[key]


def kernel(embedding_features, db_embedding, auxiliary_features):
    in_maps = make_in_maps(embedding_features, db_embedding, auxiliary_features)
    nc = get_nc()
    res = run_bass_kernel_spmd(nc, in_maps, list(range(N_CORES)))
    return np.concatenate([res.results[c]["out"] for c in range(N_CORES)],
                          axis=0).astype(np.float32)


# revision 19
# speedup vs baseline: 1.3342x; 1.0155x over previous
"""Trainium2 Bass kernel for nn_BaseEmbedder (retrieval_knn).

For each of 4096 query embeddings: find the 5 nearest of 65536 db embeddings
(Euclidean) and produce the inverse-distance-weighted sum of their auxiliary
features.  SPMD on 8 NeuronCores: queries sharded 512/core, db+aux replicated.

Per core (512 queries = 4 q-tiles of 128 partitions):
  - Scan (bf16): negS[q,j] = q.x_j - 0.5|x_j|^2 via K=34 augmented bf16
    matmuls (rows 32/33 carry the -0.5|x|^2 bias split hi/lo).  Two matmul
    streams run concurrently on PE row-groups 0 and 64 (even/odd 1024-col
    supers).
  - 4-way fold: DVE tensor_tensor(max) folds each group of four 1024-col
    supers (two pair-folds + one merge) into zfold[s] = max of the 4 values,
    16384 folded columns covering the whole db per q-tile.
  - Candidates: one max8 over zfold gives the top-8 folded values; max_index
    recovers their fold slots (needles are in-window, exact f32 match).  Each
    slot maps to FOUR db rows (the fold group); all are gathered, so no
    disambiguation is needed.
  - Exact refinement (f32): a host-prepared table
    row[g*1024+u] = [x,aux of rows (4g+m)*1024+u for m=0..3, then 4x |x|^2]
    is gathered per winning slot via per-partition indirect DMA.  Exact
    distances for all 32 candidates are recomputed on-chip; top-5 by
    threshold (5th-largest of 2*q.x - |x|^2); weights 1/(d+eps) normalized;
    weighted aux sum.

The bf16 scan only nominates candidates; all selection/weight math is exact
f32, so the result matches the f32 reference to ~1e-6.
"""

import numpy as np
import ml_dtypes

from concourse import bass, mybir
from concourse.tile import TileContext
from concourse.bass_utils import run_bass_kernel_spmd

F32 = mybir.dt.float32
BF16 = mybir.dt.bfloat16
U32 = mybir.dt.uint32
I32 = mybir.dt.int32

N_CORES = 8
NQ = 4096
NDB = 65536
D = 32
DAUG = 34   # 32 dims + bias row + bias-residual row (bf16 split)
K = 5
EPS = 1e-6

NQ_CORE = NQ // N_CORES          # 512
CHUNK = 512                      # db columns per matmul (one PSUM bank)
SUPER = 1024                     # db columns per PSUM tile / fold operand
RG_B = 64                        # partition base of the second PE row-group
PV = 260                         # paired row: 4x [x(32) aux(32)] + 4x |x|^2


def build_nc(nq_core=NQ_CORE, ndb=NDB):
    n_qt = nq_core // 128
    n_grp = ndb // (4 * SUPER)           # 4-super fold groups (16)
    fold_w = ndb // 4                    # folded columns (16384, one window)
    assert fold_w <= 16384
    ncand = 4 * 8                        # 8 needles x 4 rows per fold slot

    nc = bass.Bass()
    qT = nc.declare_dram_parameter("qT_aug", [DAUG, nq_core], BF16, isOutput=False)
    qf = nc.declare_dram_parameter("qf", [nq_core, D], F32, isOutput=False)
    qsq = nc.declare_dram_parameter("qsq", [nq_core, 1], F32, isOutput=False)
    dbT = nc.declare_dram_parameter("dbT_aug", [DAUG, ndb], BF16, isOutput=False)
    pairt = nc.declare_dram_parameter("pair_table", [ndb // 4, PV], F32,
                                      isOutput=False)
    out = nc.declare_dram_parameter("out", [nq_core, D], F32, isOutput=True)

    with TileContext(nc) as tc:
        with (
            tc.tile_pool(name="zf", bufs=1) as zfp,
            tc.tile_pool(name="db", bufs=6) as dbp,
            tc.tile_pool(name="sbA", bufs=8) as sap,
            tc.tile_pool(name="sbB", bufs=8) as sbp,
            tc.tile_pool(name="tf", bufs=2) as tfp,
            tc.tile_pool(name="psA", bufs=2, space="PSUM") as pspA,
            tc.tile_pool(name="psB", bufs=2, space="PSUM") as pspB,
            tc.tile_pool(name="sm", bufs=2) as sp,
            tc.tile_pool(name="g", bufs=1) as gp,
        ):
            for t in range(n_qt):
                # queries live on row groups 0 and RG_B so two matmul streams
                # run concurrently on the PE array
                qt = sp.tile([128, 128], BF16, tag="qt")
                nc.sync.dma_start(out=qt[0:DAUG, :],
                                  in_=qT[:, t * 128:(t + 1) * 128])
                nc.sync.dma_start(out=qt[RG_B:RG_B + DAUG, :],
                                  in_=qT[:, t * 128:(t + 1) * 128])
                qs = sp.tile([128, 1], F32, tag="qs")
                nc.sync.dma_start(out=qs[:], in_=qsq[t * 128:(t + 1) * 128, :])
                qft = sp.tile([128, D], F32, tag="qft")
                nc.sync.dma_start(out=qft[:], in_=qf[t * 128:(t + 1) * 128, :])

                candv = sp.tile([128, 8], F32, tag="candv")
                gxa = gp.tile([128, 8, PV], F32, tag="gxa")

                zfold = zfp.tile([128, fold_w], F32)
                for g in range(n_grp):
                    th = []
                    for h in range(2):
                        offa = (4 * g + 2 * h) * SUPER
                        rhs = dbp.tile([128, SUPER], BF16)
                        nc.sync.dma_start(out=rhs[0:DAUG, :],
                                          in_=dbT[:, offa:offa + SUPER])
                        nc.sync.dma_start(
                            out=rhs[RG_B:RG_B + DAUG, :],
                            in_=dbT[:, offa + SUPER:offa + 2 * SUPER])
                        psA = pspA.tile([128, SUPER], F32, tag="psA")
                        psB = pspB.tile([128, SUPER], F32, tag="psB")
                        for m in range(SUPER // CHUNK):
                            sl = slice(m * CHUNK, (m + 1) * CHUNK)
                            nc.tensor.matmul(out=psA[:, sl],
                                             lhsT=qt[0:DAUG, :],
                                             rhs=rhs[0:DAUG, sl],
                                             start=True, stop=True,
                                             tile_position=(0, 0))
                            nc.tensor.matmul(out=psB[:, sl],
                                             lhsT=qt[RG_B:RG_B + DAUG, :],
                                             rhs=rhs[RG_B:RG_B + DAUG, sl],
                                             start=True, stop=True,
                                             tile_position=(RG_B, 0))
                        sbA = sap.tile([128, SUPER], F32)
                        nc.scalar.copy(out=sbA[:], in_=psA[:])
                        sbB = sbp.tile([128, SUPER], F32)
                        nc.scalar.copy(out=sbB[:], in_=psB[:])
                        t1 = tfp.tile([128, SUPER], F32, tag=f"t{h}")
                        nc.vector.tensor_tensor(out=t1[:], in0=sbA[:],
                                                in1=sbB[:],
                                                op=mybir.AluOpType.max)
                        th.append(t1)
                    nc.vector.tensor_tensor(
                        out=zfold[:, g * SUPER:(g + 1) * SUPER],
                        in0=th[0][:], in1=th[1][:], op=mybir.AluOpType.max)
                # top-8 folded values + their fold slots (one window = all db)
                w8 = candv[:, 0:8]
                nc.vector.max(out=w8, in_=zfold[:])
                pos = sp.tile([128, 8], U32, tag="pos")
                nc.vector.max_index(out=pos[:], in_max=w8, in_values=zfold[:])
                ji = sp.tile([128, 8], I32, tag="ji")
                nc.vector.tensor_copy(ji[:], pos[:])
                for i in range(8):
                    nc.gpsimd.indirect_dma_start(
                        out=gxa[:, i, :], out_offset=None, in_=pairt[:],
                        in_offset=bass.IndirectOffsetOnAxis(
                            ap=ji[:, i:i + 1], axis=0))

                # ---- exact f32 refinement over the 32 candidates ----
                # gxa row: 4x [x(32) aux(32)] then 4x |x|^2
                base = gxa[:, :, 0:8 * D].rearrange("p c (h v) -> p c h v", h=4)
                gx = base[:, :, :, 0:D]
                ga = base[:, :, :, D:2 * D]
                xsq = gxa[:, :, 8 * D:8 * D + 4]          # [128, 8, 4]
                # dots[q, c, h] = q . x
                pr = gp.tile([128, 8, 4, D], F32, tag="pr")
                nc.vector.tensor_tensor(
                    out=pr[:], in0=gx,
                    in1=qft[:].unsqueeze(1).unsqueeze(1)
                              .to_broadcast([128, 8, 4, D]),
                    op=mybir.AluOpType.mult)
                dots = sp.tile([128, 8, 4], F32, tag="dots")
                nc.vector.tensor_reduce(out=dots[:], in_=pr[:],
                                        axis=mybir.AxisListType.X,
                                        op=mybir.AluOpType.add)
                # neg2 = 2*dots - xsq  (dsq = qsq - neg2)
                neg2 = sp.tile([128, ncand], F32, tag="neg2")
                nc.vector.scalar_tensor_tensor(
                    out=neg2[:].rearrange("p (c h) -> p c h", h=4),
                    in0=dots[:], scalar=2.0, in1=xsq,
                    op0=mybir.AluOpType.mult, op1=mybir.AluOpType.subtract)
                t8 = sp.tile([128, 8], F32, tag="t8")
                nc.vector.max(out=t8[:], in_=neg2[:])
                mask = sp.tile([128, ncand], F32, tag="mask")
                nc.vector.tensor_scalar(mask[:], neg2[:], t8[:, 4:5], None,
                                        op0=mybir.AluOpType.is_ge)
                dsq = sp.tile([128, ncand], F32, tag="dsq")
                nc.vector.tensor_scalar(dsq[:], neg2[:], -1.0, qs[:, 0:1],
                                        op0=mybir.AluOpType.mult,
                                        op1=mybir.AluOpType.add)
                nc.vector.tensor_scalar_max(dsq[:], dsq[:], 0.0)
                dist = sp.tile([128, ncand], F32, tag="dist")
                nc.scalar.sqrt(out=dist[:], in_=dsq[:])
                nc.vector.tensor_scalar_add(dist[:], dist[:], EPS)
                rec = sp.tile([128, ncand], F32, tag="rec")
                nc.vector.reciprocal(out=rec[:], in_=dist[:])
                wgt = sp.tile([128, ncand], F32, tag="wgt")
                nc.vector.tensor_tensor(out=wgt[:], in0=rec[:], in1=mask[:],
                                        op=mybir.AluOpType.mult)
                wsum = sp.tile([128, 1], F32, tag="wsum")
                nc.vector.tensor_reduce(out=wsum[:], in_=wgt[:],
                                        axis=mybir.AxisListType.X,
                                        op=mybir.AluOpType.add)
                winv = sp.tile([128, 1], F32, tag="winv")
                nc.vector.reciprocal(out=winv[:], in_=wsum[:])

                # weighted sum of gathered aux rows
                prod = gp.tile([128, 8, 4, D], F32, tag="prod")
                nc.vector.tensor_tensor(
                    out=prod[:], in0=ga,
                    in1=wgt[:].rearrange("p (c h) -> p c h", h=4).unsqueeze(-1)
                              .to_broadcast([128, 8, 4, D]),
                    op=mybir.AluOpType.mult)
                acc = sp.tile([128, D], F32, tag="accr")
                nc.vector.tensor_reduce(
                    out=acc[:],
                    in_=prod[:].rearrange("p i h a -> p a (i h)"),
                    axis=mybir.AxisListType.X, op=mybir.AluOpType.add)
                outt = sp.tile([128, D], F32, tag="outt")
                nc.vector.tensor_scalar(outt[:], acc[:], winv[:, 0:1], None,
                                        op0=mybir.AluOpType.mult)
                nc.sync.dma_start(out=out[t * 128:(t + 1) * 128, :], in_=outt[:])

    split_multi_waits(nc)
    return nc


def split_multi_waits(nc):
    """The walrus build in this container supports a single sync-wait per
    instruction; Tile's tail drain carries one wait per live proc.  Split
    any multi-wait instruction into single-wait NoOps ahead of it."""
    for f in nc.m.functions:
        for blk in f.blocks:
            newinsts = []
            for ins in blk.instructions:
                si = ins.sync_info
                if si is not None and si.on_wait and len(si.on_wait) > 1:
                    waits = list(si.on_wait)
                    for k, w in enumerate(waits[:-1]):
                        nop = mybir.InstNoOp(name=f"{ins.name}-ws{k}", ins=[],
                                             outs=[])
                        nop.engine = ins.engine
                        nop.sync_info = mybir.SyncInfo(on_wait=[w], on_update=[])
                        newinsts.append(nop)
                    ins.sync_info = mybir.SyncInfo(on_wait=[waits[-1]],
                                                   on_update=list(si.on_update))
                newinsts.append(ins)
            blk.instructions = newinsts


def make_in_maps(embedding_features, db_embedding, auxiliary_features):
    q = np.ascontiguousarray(np.asarray(embedding_features, dtype=np.float32))
    db = np.ascontiguousarray(np.asarray(db_embedding, dtype=np.float32))
    aux = np.ascontiguousarray(np.asarray(auxiliary_features, dtype=np.float32))
    ndb = db.shape[0]
    nq_core = q.shape[0] // N_CORES
    bf = ml_dtypes.bfloat16
    bias = -0.5 * (db * db).sum(1)                      # exact f32
    b_hi = bias.astype(bf).astype(np.float32)
    b_lo = (bias - b_hi).astype(bf)
    dbT_aug = np.ascontiguousarray(np.concatenate(
        [db.T.astype(bf), b_hi.astype(bf)[None, :], b_lo[None, :]], axis=0,
        dtype=bf))
    # paired table: fold slot s = g*S + u covers db rows (4g+m)*S + u, m=0..3
    idx = np.arange(ndb // 4)
    g_i = idx // SUPER
    u_i = idx % SUPER
    dbsq = (db * db).sum(1)
    pair_table = np.zeros((ndb // 4, PV), np.float32)
    for m in range(4):
        jm = (4 * g_i + m) * SUPER + u_i
        pair_table[:, 2 * m * D:(2 * m + 1) * D] = db[jm]
        pair_table[:, (2 * m + 1) * D:(2 * m + 2) * D] = aux[jm]
        pair_table[:, 8 * D + m] = dbsq[jm]
    pair_table = np.ascontiguousarray(pair_table)
    in_maps = []
    for c in range(N_CORES):
        qs = q[c * nq_core:(c + 1) * nq_core]
        qT_aug = np.ascontiguousarray(np.concatenate(
            [qs.T.astype(bf), np.ones((2, nq_core), bf)], axis=0, dtype=bf))
        qsq = np.ascontiguousarray((qs * qs).sum(1).reshape(nq_core, 1)
                                   ).astype(np.float32)
        in_maps.append({"qT_aug": qT_aug, "qf": qs, "qsq": qsq,
                        "dbT_aug": dbT_aug, "pair_table": pair_table})
    return in_maps


_NC_CACHE = {}


def get_nc(nq_core=NQ_CORE, ndb=NDB):
    key = (nq_core, ndb)
    if key not in _NC_CACHE:
        _NC_CACHE[key] = build_nc(nq_core, ndb)
    return _NC_CACHE[key]


def kernel(embedding_features, db_embedding, auxiliary_features):
    in_maps = make_in_maps(embedding_features, db_embedding, auxiliary_features)
    nc = get_nc()
    res = run_bass_kernel_spmd(nc, in_maps, list(range(N_CORES)))
    return np.concatenate([res.results[c]["out"] for c in range(N_CORES)],
                          axis=0).astype(np.float32)
